# revision 39
# baseline (speedup 1.0000x reference)
"""CAM (channel attention) module kernel for Trainium2, 8 NeuronCores.

Reference computation (per sample, x: [C, N] with C=512, N=64*64):
    energy    = x @ x.T                      # [C, C] Gram matrix
    att       = softmax(rowmax(energy) - energy, axis=-1)
    out       = gamma * (att @ x) + x

softmax(rowmax - e) == softmax(-e); stabilized with the row-min m_i:
att[i,j] = exp(m_i - e_ij) / S_i.

Sharding: pure data parallel over batch B=16 -> 2 samples per core.

v5 pipeline (attention branch fp8e4 + DoubleRow; epilogue exact f32,
emitted f16), software-pipelined across samples:
  per sample s: loads -> ACT/gpsimd casts to fp8 pair tiles m2m
  [128, 2, N] -> {PE transposes of s INTERLEAVED with mm2+epilogue of
  s-1} -> triangular mm1 panels (DoubleRow, K=256/instr) with mirrored
  symmetric blocks -> softmax (DVE rowmin, ACT exp with fused row-sum)
  -> PT = P.T @ blockdiag(gamma/S) via DoubleRow -> (mm2 deferred into
  sample s+1's transpose phase; last sample's mm2 runs at the tail)

The interleave keeps the PE queue saturated at the sample boundary
(the HAM clock governor halves the PE clock after idle gaps) and lets
the DVE-only epilogue (gpsimd cannot read PSUM) pace against transpose
work instead of gating mm2.

gamma=0 path is exact: D underflows to 0 in fp8 -> psum = 0 -> out =
f16(x) (~2^-11 max relative error from the f16 store).
"""

import numpy as np

import concourse.bacc as bacc
import concourse.tile as tile
from concourse import mybir
from concourse.bass_utils import run_bass_kernel_spmd
from concourse.masks import make_identity

B, C, H, W = 16, 512, 64, 64
N = H * W
NCORES = 8
BPC = B // NCORES  # samples per core
CB = C // 128      # channel blocks (4)
NK = N // 128      # 128-wide n-chunks (32)
NP = NK // 2       # n-chunk pairs (16)
NT = N // 512      # 512-wide n-tiles (8)

F32 = mybir.dt.float32
F16 = mybir.dt.float16
FP8 = mybir.dt.float8e4
DR = mybir.MatmulPerfMode.DoubleRow
Copy = mybir.ActivationFunctionType.Copy


def _emit(nc, tc, ctx, x, gamma, out):
    consts = ctx.enter_context(tc.tile_pool(name="consts", bufs=1))
    nat_pool = ctx.enter_context(tc.tile_pool(name="nat", bufs=2 * CB))
    m2m_pool = ctx.enter_context(tc.tile_pool(name="m2m", bufs=4))
    xt_pool = ctx.enter_context(tc.tile_pool(name="xt", bufs=NP))
    pp_pool = ctx.enter_context(tc.tile_pool(name="pp", bufs=2))
    ptp_pool = ctx.enter_context(tc.tile_pool(name="ptp", bufs=2))
    dd_pool = ctx.enter_context(tc.tile_pool(name="dd", bufs=2))
    eblk_pool = ctx.enter_context(tc.tile_pool(name="eblk", bufs=6))
    small = ctx.enter_context(tc.tile_pool(name="small", bufs=4 * CB))
    outs_pool = ctx.enter_context(tc.tile_pool(name="outs", bufs=2))
    tmp_pool = ctx.enter_context(tc.tile_pool(name="tmp", bufs=2))
    psum_e = ctx.enter_context(tc.tile_pool(name="psum_e", bufs=3, space="PSUM"))
    psum_g = ctx.enter_context(tc.tile_pool(name="psum_g", bufs=5, space="PSUM"))

    # wcon8 only feeds dependency-free warm matmuls: memset it so the
    # first warm matmul needs nothing from the identity chain
    wcon8 = consts.tile([128, 128], FP8)
    nc.vector.memset(wcon8[:], 0.25)
    identity = consts.tile([128, 128], F32)
    make_identity(nc, identity[:])
    id8 = consts.tile([128, 128], FP8)
    nc.vector.tensor_copy(out=id8[:], in_=identity[:])
    g_sb = consts.tile([128, 1], F32)
    nc.gpsimd.dma_start(out=g_sb[:], in_=gamma[:].to_broadcast((128, 1)))

    # persistent paired block-diag D tiles: zero quadrants memset once;
    # only the gamma/S diagonal quadrants are rewritten per sample
    dd = [
        dd_pool.tile([128, 2, 256], FP8, tag="dd", name=f"dd_{t}")
        for t in range(CB // 2)
    ]
    for t in range(CB // 2):
        nc.gpsimd.memset(dd[t][:], 0.0)

    QN = N // 8

    # state of the previous sample, whose mm2 is deferred into the
    # current sample's transpose phase
    pend = {}

    def emit_mm2_group(g, tail=False):
        """One mm2 psum group: ci = g//NT, nt = g%NT (ci-outer so nat
        chunks free early for the next sample's loads). In the tail
        (no transpose work to interleave) the DVE-only epilogue is the
        pace-setter, so some groups route via ACT-copy + gpsimd-add
        and stores spill to the idle sync queue."""
        s, m2m, ptp, nat = pend["s"], pend["m2m"], pend["ptp"], pend["nat"]
        ci, nt = g // NT, g % NT
        if nt == 0:
            pend["o_row"] = outs_pool.tile(
                [128, N], F16, tag="o", name=f"orow{s}_{ci}"
            )
        ops = psum_g.tile([128, 512], F32, tag="g")
        for t in range(CB // 2):
            nc.tensor.matmul(
                ops[:],
                ptp[t][:, :, 128 * ci : 128 * (ci + 1)],
                m2m[t][:, :, 512 * nt : 512 * (nt + 1)],
                start=(t == 0),
                stop=(t == CB // 2 - 1),
                perf_mode=DR,
            )
        o_slice = pend["o_row"][:, 512 * nt : 512 * (nt + 1)]
        nat_slice = nat[ci][:, 512 * nt : 512 * (nt + 1)]
        if tail and nt % 4 == 1:
            t_sb = tmp_pool.tile([128, 512], F32, tag="tmp")
            nc.scalar.activation(
                out=t_sb[:], in_=ops[:], func=Copy, bias=0.0, scale=1.0,
            )
            nc.gpsimd.tensor_add(out=o_slice, in0=t_sb[:], in1=nat_slice)
        else:
            nc.vector.scalar_tensor_tensor(
                out=o_slice, in0=ops[:], scalar=1.0, in1=nat_slice,
                op0=mybir.AluOpType.bypass, op1=mybir.AluOpType.add,
            )
        if tail:
            # piecewise stores: the final DMA issues right after the
            # last epilogue, shrinking the end-of-kernel drain
            eng = nc.sync if nt % 2 == 1 else nc.gpsimd
            eng.dma_start(
                out=out[
                    s, 128 * ci : 128 * (ci + 1), 512 * nt : 512 * (nt + 1)
                ],
                in_=o_slice,
            )
        elif nt == NT - 1:
            nc.gpsimd.dma_start(
                out=out[s, 128 * ci : 128 * (ci + 1), :],
                in_=pend["o_row"][:],
            )

    for s in range(BPC):
        # ---- load natural layout in 8 interleaved column pieces ----
        nat = [
            nat_pool.tile([128, N], F32, tag="nat", name=f"nat{s}_{c}")
            for c in range(CB)
        ]
        for q in range(8):
            for c in range(CB):
                nc.sync.dma_start(
                    out=nat[c][:, QN * q : QN * (q + 1)],
                    in_=x[s, 128 * c : 128 * (c + 1), QN * q : QN * (q + 1)],
                )

        # ---- fp8 pair tiles; cast emission is deferred piecewise into
        # the transpose loop so ACT alternates casts and psum drains
        # instead of queueing all casts ahead of the drains ----
        m2m = [
            m2m_pool.tile([128, 2, N], FP8, tag="m2m", name=f"m2m{s}_{t}")
            for t in range(CB // 2)
        ]

        def emit_casts(q):
            for c in range(CB):
                if (c, q) in ((3, 4), (3, 6), (2, 5), (2, 7)):
                    nc.gpsimd.tensor_copy(
                        out=m2m[c // 2][:, c % 2, QN * q : QN * (q + 1)],
                        in_=nat[c][:, QN * q : QN * (q + 1)],
                    )
                else:
                    nc.scalar.activation(
                        out=m2m[c // 2][:, c % 2, QN * q : QN * (q + 1)],
                        in_=nat[c][:, QN * q : QN * (q + 1)],
                        func=Copy, bias=0.0, scale=1.0,
                    )

        emit_casts(0)
        emit_casts(1)

        # keep the PE busy (HAM warm) while the first pieces land
        warm_ps = psum_g.tile([128, 128], F32, tag="g", name=f"warm{s}")
        nwarm = 16 if s == 0 else 4
        for w in range(nwarm):
            nc.tensor.matmul(warm_ps[:], wcon8[:], wcon8[:], start=(w == 0), stop=False)
        nc.tensor.matmul(warm_ps[:], wcon8[:], wcon8[:], start=False, stop=True)

        def warm_fill(wtag, n):
            # dependency-free PE filler: keeps the HAM duty cycle high
            # through phases whose real work has cross-engine bubbles
            wps = psum_g.tile([128, 128], F32, tag="g", name=wtag)
            for w in range(n):
                nc.tensor.matmul(
                    wps[:], wcon8[:], wcon8[:], start=(w == 0), stop=(w == n - 1)
                )

        # ---- transposes (fp8, step-2 psum) + mm1 panel ci=0,
        #      interleaved with the previous sample's mm2 ----
        xts = []
        e_ps = [None] * CB
        e_ps[0] = psum_e.tile([128, C], F32, tag="e", name=f"e_ps{s}_0")

        def mm1_ci0(qp):
            nc.tensor.matmul(
                e_ps[0][:],
                xts[qp][:, :, 0:128],
                xts[qp][:, :, 0:C],
                start=(qp == 0),
                stop=(qp == NP - 1),
                perf_mode=DR,
            )

        for qp in range(NP):
            # release the next cast piece just ahead of its consumers
            if qp % 2 == 0 and qp // 2 + 2 < 8:
                emit_casts(qp // 2 + 2)
            xt = xt_pool.tile([128, 2, C], FP8, tag="xt", name=f"xt{s}_{qp}")
            for half in range(2):
                k = 2 * qp + half
                t_ps = psum_g.tile([128, C, 2], FP8, tag="g", name=f"tps{s}_{k}")
                for c in range(CB):
                    nc.tensor.transpose(
                        t_ps[:, 128 * c : 128 * (c + 1), 0],
                        m2m[c // 2][:, c % 2, 128 * k : 128 * (k + 1)],
                        id8[:],
                    )
                if k % 2 == 0:
                    nc.vector.tensor_copy(out=xt[:, half, :], in_=t_ps[:, :, 0])
                else:
                    nc.scalar.activation(
                        out=xt[:, half, :], in_=t_ps[:, :, 0],
                        func=Copy, bias=0.0, scale=1.0,
                    )
            xts.append(xt)
            if qp >= 1:
                mm1_ci0(qp - 1)
            if pend:
                # stagger past the ptt drain: no mm2 on the first pair
                if qp >= 1:
                    emit_mm2_group(2 * (qp - 1))
                    emit_mm2_group(2 * (qp - 1) + 1)
            else:
                # sample 0 has no mm2 to interleave: pad PE duty
                warm_fill(f"wfill{s}_{qp}", 3)
        if pend:
            emit_mm2_group(30)
            emit_mm2_group(31)
        mm1_ci0(NP - 1)
        pend.clear()

        # ---- mm1 panels ci=1..3 (triangular) + mirror stash/restore ----
        e_blk = {}

        def stash(ci):
            for cj in range(ci + 1, CB):
                blk = eblk_pool.tile(
                    [128, 128], F32, tag="eblk", name=f"eblk{s}_{ci}_{cj}"
                )
                nc.vector.tensor_copy(
                    out=blk[:], in_=e_ps[ci][:, 128 * cj : 128 * (cj + 1)]
                )
                e_blk[(ci, cj)] = blk

        stash(0)
        for ci in range(1, CB):
            lo = 128 * ci
            e_ps[ci] = psum_e.tile([128, C], F32, tag="e", name=f"e_ps{s}_{ci}")
            for qp in range(NP):
                nc.tensor.matmul(
                    e_ps[ci][:, lo:C],
                    xts[qp][:, :, lo : lo + 128],
                    xts[qp][:, :, lo:C],
                    start=(qp == 0),
                    stop=(qp == NP - 1),
                    perf_mode=DR,
                )
            stash(ci)
            for cj in range(ci):
                nc.tensor.matmul(
                    e_ps[ci][:, 128 * cj : 128 * (cj + 1)],
                    e_blk[(cj, ci)][:],
                    identity[:],
                    is_transpose=True,
                )
            warm_fill(f"wpan{s}_{ci}", 2)

        # ---- softmax: P = exp(m - e) fp8, S = rowsum ----
        pp = [
            pp_pool.tile([128, 2, C], FP8, tag="pp", name=f"pp{s}_{t}")
            for t in range(CB // 2)
        ]
        rs = []
        for ci in range(CB):
            m = small.tile([128, 1], F32, tag="m")
            nc.vector.tensor_reduce(
                out=m[:], in_=e_ps[ci][:], axis=mybir.AxisListType.X,
                op=mybir.AluOpType.min,
            )
            ssum = small.tile([128, 1], F32, tag="s")
            nc.scalar.activation(
                out=pp[ci // 2][:, ci % 2, :],
                in_=e_ps[ci][:],
                func=mybir.ActivationFunctionType.Exp,
                bias=m[:], scale=-1.0, accum_out=ssum[:],
            )
            r = small.tile([128, 1], F32, tag="r")
            nc.vector.reciprocal(out=r[:], in_=ssum[:])
            rs.append(r)

        # rewrite diagonal quadrants: d = (I * (1/S)) * gamma in fp8
        for ci in range(CB):
            nc.vector.tensor_scalar(
                out=dd[ci // 2][:, ci % 2, 128 * (ci % 2) : 128 * (ci % 2) + 128],
                in0=identity[:],
                scalar1=rs[ci][:],
                scalar2=g_sb[:],
                op0=mybir.AluOpType.mult,
                op1=mybir.AluOpType.mult,
            )

        # ---- PT = P.T @ D via DoubleRow: PT[j, i] = gamma * att[i, j] ----
        ptps = [
            psum_g.tile([128, C], F32, tag="g", name=f"ptp{s}_{bj}")
            for bj in range(CB)
        ]
        for t in range(CB // 2):
            for bj in range(CB):
                nc.tensor.matmul(
                    ptps[bj][:, 256 * t : 256 * (t + 1)],
                    pp[t][:, :, 128 * bj : 128 * (bj + 1)],
                    dd[t][:, :, 0:256],
                    start=True,
                    stop=True,
                    perf_mode=DR,
                )
        ptp = [
            ptp_pool.tile([128, 2, C], FP8, tag="pt", name=f"ptp8{s}_{t}")
            for t in range(CB // 2)
        ]
        for bj in range(CB):
            if bj % 2 == 0:
                nc.vector.tensor_copy(out=ptp[bj // 2][:, bj % 2, :], in_=ptps[bj][:])
            else:
                nc.scalar.activation(
                    out=ptp[bj // 2][:, bj % 2, :], in_=ptps[bj][:],
                    func=Copy, bias=0.0, scale=1.0,
                )

        pend.update({"s": s, "m2m": m2m, "ptp": ptp, "nat": nat})

    # ---- tail: the last sample's mm2 ----
    wps = psum_g.tile([128, 128], F32, tag="g", name="wtail")
    for w in range(6):
        nc.tensor.matmul(wps[:], wcon8[:], wcon8[:], start=(w == 0), stop=(w == 5))
    for g in range(CB * NT):
        emit_mm2_group(g, tail=True)
    pend.clear()


_NC_CACHE = None


def _build():
    global _NC_CACHE
    if _NC_CACHE is not None:
        return _NC_CACHE
    from contextlib import ExitStack

    nc = bacc.Bacc("TRN2", target_bir_lowering=False)
    x = nc.dram_tensor("x", [BPC, C, N], F32, kind="ExternalInput")
    gamma = nc.dram_tensor("gamma", [1, 1], F32, kind="ExternalInput")
    out = nc.dram_tensor("out", [BPC, C, N], F16, kind="ExternalOutput")
    with tile.TileContext(nc) as tc:
        with ExitStack() as ctx:
            _emit(nc, tc, ctx, x[:], gamma[:], out[:])
    nc.compile()
    _NC_CACHE = nc
    return nc


def kernel(x, gamma):
    x = np.ascontiguousarray(np.asarray(x, dtype=np.float32))
    gamma = np.ascontiguousarray(np.asarray(gamma, dtype=np.float32))
    assert x.shape == (B, C, H, W), x.shape
    xf = x.reshape(B, C, N)
    nc = _build()
    in_maps = [
        {
            "x": xf[c * BPC : (c + 1) * BPC],
            "gamma": gamma.reshape(1, 1),
        }
        for c in range(NCORES)
    ]
    res = run_bass_kernel_spmd(nc, in_maps, core_ids=list(range(NCORES)))
    out = np.concatenate(
        [np.asarray(res.results[c]["out"]) for c in range(NCORES)], axis=0
    )
    return out.astype(np.float32).reshape(B, C, H, W)


# revision 40
# speedup vs baseline: 1.0246x; 1.0246x over previous
"""CAM (channel attention) module kernel for Trainium2, 8 NeuronCores.

Reference computation (per sample, x: [C, N] with C=512, N=64*64):
    energy    = x @ x.T                      # [C, C] Gram matrix
    att       = softmax(rowmax(energy) - energy, axis=-1)
    out       = gamma * (att @ x) + x

softmax(rowmax - e) == softmax(-e); stabilized with the row-min m_i:
att[i,j] = exp(m_i - e_ij) / S_i.

Sharding: pure data parallel over batch B=16 -> 2 samples per core.

v5 pipeline (attention branch fp8e4 + DoubleRow; epilogue exact f32,
emitted f16), software-pipelined across samples:
  per sample s: loads -> ACT/gpsimd casts to fp8 pair tiles m2m
  [128, 2, N] -> {PE transposes of s INTERLEAVED with mm2+epilogue of
  s-1} -> triangular mm1 panels (DoubleRow, K=256/instr) with mirrored
  symmetric blocks -> softmax (DVE rowmin, ACT exp with fused row-sum)
  -> PT = P.T @ blockdiag(gamma/S) via DoubleRow -> (mm2 deferred into
  sample s+1's transpose phase; last sample's mm2 runs at the tail)

The interleave keeps the PE queue saturated at the sample boundary
(the HAM clock governor halves the PE clock after idle gaps) and lets
the DVE-only epilogue (gpsimd cannot read PSUM) pace against transpose
work instead of gating mm2.

gamma=0 path is exact: D underflows to 0 in fp8 -> psum = 0 -> out =
f16(x) (~2^-11 max relative error from the f16 store).
"""

import numpy as np

import concourse.bacc as bacc
import concourse.tile as tile
from concourse import mybir
from concourse.bass_utils import run_bass_kernel_spmd
from concourse.masks import make_identity

B, C, H, W = 16, 512, 64, 64
N = H * W
NCORES = 8
BPC = B // NCORES  # samples per core
CB = C // 128      # channel blocks (4)
NK = N // 128      # 128-wide n-chunks (32)
NP = NK // 2       # n-chunk pairs (16)
NT = N // 512      # 512-wide n-tiles (8)

F32 = mybir.dt.float32
F16 = mybir.dt.float16
FP8 = mybir.dt.float8e4
DR = mybir.MatmulPerfMode.DoubleRow
Copy = mybir.ActivationFunctionType.Copy


def _emit(nc, tc, ctx, x, gamma, out):
    consts = ctx.enter_context(tc.tile_pool(name="consts", bufs=1))
    nat_pool = ctx.enter_context(tc.tile_pool(name="nat", bufs=2 * CB))
    m2m_pool = ctx.enter_context(tc.tile_pool(name="m2m", bufs=4))
    xt_pool = ctx.enter_context(tc.tile_pool(name="xt", bufs=NP))
    pp_pool = ctx.enter_context(tc.tile_pool(name="pp", bufs=2))
    ptp_pool = ctx.enter_context(tc.tile_pool(name="ptp", bufs=2))
    dd_pool = ctx.enter_context(tc.tile_pool(name="dd", bufs=2))
    eblk_pool = ctx.enter_context(tc.tile_pool(name="eblk", bufs=6))
    small = ctx.enter_context(tc.tile_pool(name="small", bufs=4 * CB))
    outs_pool = ctx.enter_context(tc.tile_pool(name="outs", bufs=2))
    tmp_pool = ctx.enter_context(tc.tile_pool(name="tmp", bufs=2))
    psum_e = ctx.enter_context(tc.tile_pool(name="psum_e", bufs=3, space="PSUM"))
    psum_g = ctx.enter_context(tc.tile_pool(name="psum_g", bufs=5, space="PSUM"))

    # wcon8 only feeds dependency-free warm matmuls: memset it so the
    # first warm matmul needs nothing from the identity chain
    wcon8 = consts.tile([128, 128], FP8)
    nc.vector.memset(wcon8[:], 0.25)
    identity = consts.tile([128, 128], F32)
    make_identity(nc, identity[:])
    id8 = consts.tile([128, 128], FP8)
    nc.vector.tensor_copy(out=id8[:], in_=identity[:])
    g_sb = consts.tile([128, 1], F32)
    nc.gpsimd.dma_start(out=g_sb[:], in_=gamma[:].to_broadcast((128, 1)))

    # persistent paired block-diag D tiles: zero quadrants memset once;
    # only the gamma/S diagonal quadrants are rewritten per sample
    dd = [
        dd_pool.tile([128, 2, 256], FP8, tag="dd", name=f"dd_{t}")
        for t in range(CB // 2)
    ]
    for t in range(CB // 2):
        nc.gpsimd.memset(dd[t][:], 0.0)

    QN = N // 8

    # state of the previous sample, whose mm2 is deferred into the
    # current sample's transpose phase
    pend = {}

    def emit_mm2_group(g, tail=False):
        """One mm2 psum group: ci = g//NT, nt = g%NT (ci-outer so nat
        chunks free early for the next sample's loads). In the tail
        (no transpose work to interleave) the DVE-only epilogue is the
        pace-setter, so some groups route via ACT-copy + gpsimd-add
        and stores spill to the idle sync queue."""
        s, m2m, ptp, nat = pend["s"], pend["m2m"], pend["ptp"], pend["nat"]
        ci, nt = g // NT, g % NT
        if nt == 0:
            pend["o_row"] = outs_pool.tile(
                [128, N], F16, tag="o", name=f"orow{s}_{ci}"
            )
        ops = psum_g.tile([128, 512], F32, tag="g")
        for t in range(CB // 2):
            nc.tensor.matmul(
                ops[:],
                ptp[t][:, :, 128 * ci : 128 * (ci + 1)],
                m2m[t][:, :, 512 * nt : 512 * (nt + 1)],
                start=(t == 0),
                stop=(t == CB // 2 - 1),
                perf_mode=DR,
            )
        o_slice = pend["o_row"][:, 512 * nt : 512 * (nt + 1)]
        nat_slice = nat[ci][:, 512 * nt : 512 * (nt + 1)]
        if tail and nt % 4 == 1:
            t_sb = tmp_pool.tile([128, 512], F32, tag="tmp")
            nc.scalar.activation(
                out=t_sb[:], in_=ops[:], func=Copy, bias=0.0, scale=1.0,
            )
            nc.gpsimd.tensor_add(out=o_slice, in0=t_sb[:], in1=nat_slice)
        else:
            nc.vector.scalar_tensor_tensor(
                out=o_slice, in0=ops[:], scalar=1.0, in1=nat_slice,
                op0=mybir.AluOpType.bypass, op1=mybir.AluOpType.add,
            )
        if tail:
            # piecewise stores: the final DMA issues right after the
            # last epilogue, shrinking the end-of-kernel drain
            eng = nc.sync if nt % 2 == 1 else nc.gpsimd
            eng.dma_start(
                out=out[
                    s, 128 * ci : 128 * (ci + 1), 512 * nt : 512 * (nt + 1)
                ],
                in_=o_slice,
            )
        elif nt == NT - 1:
            nc.gpsimd.dma_start(
                out=out[s, 128 * ci : 128 * (ci + 1), :],
                in_=pend["o_row"][:],
            )

    for s in range(BPC):
        # ---- load natural layout in 8 interleaved column pieces ----
        nat = [
            nat_pool.tile([128, N], F32, tag="nat", name=f"nat{s}_{c}")
            for c in range(CB)
        ]
        for q in range(8):
            for c in range(CB):
                nc.sync.dma_start(
                    out=nat[c][:, QN * q : QN * (q + 1)],
                    in_=x[s, 128 * c : 128 * (c + 1), QN * q : QN * (q + 1)],
                )

        # ---- fp8 pair tiles; cast emission is deferred piecewise into
        # the transpose loop so ACT alternates casts and psum drains
        # instead of queueing all casts ahead of the drains ----
        m2m = [
            m2m_pool.tile([128, 2, N], FP8, tag="m2m", name=f"m2m{s}_{t}")
            for t in range(CB // 2)
        ]

        def emit_casts(q):
            for c in range(CB):
                if (c, q) in ((3, 4), (3, 6), (2, 5), (2, 7)):
                    nc.gpsimd.tensor_copy(
                        out=m2m[c // 2][:, c % 2, QN * q : QN * (q + 1)],
                        in_=nat[c][:, QN * q : QN * (q + 1)],
                    )
                else:
                    nc.scalar.activation(
                        out=m2m[c // 2][:, c % 2, QN * q : QN * (q + 1)],
                        in_=nat[c][:, QN * q : QN * (q + 1)],
                        func=Copy, bias=0.0, scale=1.0,
                    )

        emit_casts(0)
        emit_casts(1)

        # keep the PE busy (HAM warm) while the first pieces land
        warm_ps = psum_g.tile([128, 128], F32, tag="g", name=f"warm{s}")
        nwarm = 16 if s == 0 else 4
        for w in range(nwarm):
            nc.tensor.matmul(warm_ps[:], wcon8[:], wcon8[:], start=(w == 0), stop=False)
        nc.tensor.matmul(warm_ps[:], wcon8[:], wcon8[:], start=False, stop=True)

        def warm_fill(wtag, n):
            # dependency-free PE filler: keeps the HAM duty cycle high
            # through phases whose real work has cross-engine bubbles
            wps = psum_g.tile([128, 128], F32, tag="g", name=wtag)
            for w in range(n):
                nc.tensor.matmul(
                    wps[:], wcon8[:], wcon8[:], start=(w == 0), stop=(w == n - 1)
                )

        # ---- transposes (fp8, step-2 psum) + mm1 panel ci=0,
        #      interleaved with the previous sample's mm2 ----
        xts = []
        e_ps = [None] * CB
        e_ps[0] = psum_e.tile([128, C], F32, tag="e", name=f"e_ps{s}_0")

        def mm1_ci0(qp):
            nc.tensor.matmul(
                e_ps[0][:],
                xts[qp][:, :, 0:128],
                xts[qp][:, :, 0:C],
                start=(qp == 0),
                stop=(qp == NP - 1),
                perf_mode=DR,
            )

        for qp in range(NP):
            # release the next cast piece just ahead of its consumers
            if qp % 2 == 0 and qp // 2 + 2 < 8:
                emit_casts(qp // 2 + 2)
            xt = xt_pool.tile([128, 2, C], FP8, tag="xt", name=f"xt{s}_{qp}")
            for half in range(2):
                k = 2 * qp + half
                t_ps = psum_g.tile([128, C, 2], FP8, tag="g", name=f"tps{s}_{k}")
                for c in range(CB):
                    nc.tensor.transpose(
                        t_ps[:, 128 * c : 128 * (c + 1), 0],
                        m2m[c // 2][:, c % 2, 128 * k : 128 * (k + 1)],
                        id8[:],
                    )
                if k % 2 == 0:
                    nc.vector.tensor_copy(out=xt[:, half, :], in_=t_ps[:, :, 0])
                else:
                    nc.scalar.activation(
                        out=xt[:, half, :], in_=t_ps[:, :, 0],
                        func=Copy, bias=0.0, scale=1.0,
                    )
            xts.append(xt)
            if qp >= 1:
                mm1_ci0(qp - 1)
            if pend:
                # stagger past the ptt drain: no mm2 on the first pair
                if qp >= 1:
                    emit_mm2_group(2 * (qp - 1))
                    emit_mm2_group(2 * (qp - 1) + 1)
            else:
                # sample 0 has no mm2 to interleave: pad PE duty
                warm_fill(f"wfill{s}_{qp}", 3)
        if pend:
            emit_mm2_group(30)
            emit_mm2_group(31)
        mm1_ci0(NP - 1)
        pend.clear()

        # ---- mm1 panels ci=1..3 (triangular) + mirror stash/restore ----
        e_blk = {}

        def stash(ci):
            for cj in range(ci + 1, CB):
                blk = eblk_pool.tile(
                    [128, 128], F32, tag="eblk", name=f"eblk{s}_{ci}_{cj}"
                )
                nc.vector.tensor_copy(
                    out=blk[:], in_=e_ps[ci][:, 128 * cj : 128 * (cj + 1)]
                )
                e_blk[(ci, cj)] = blk

        stash(0)
        for ci in range(1, CB):
            lo = 128 * ci
            e_ps[ci] = psum_e.tile([128, C], F32, tag="e", name=f"e_ps{s}_{ci}")
            for qp in range(NP):
                nc.tensor.matmul(
                    e_ps[ci][:, lo:C],
                    xts[qp][:, :, lo : lo + 128],
                    xts[qp][:, :, lo:C],
                    start=(qp == 0),
                    stop=(qp == NP - 1),
                    perf_mode=DR,
                )
            stash(ci)
            for cj in range(ci):
                nc.tensor.matmul(
                    e_ps[ci][:, 128 * cj : 128 * (cj + 1)],
                    e_blk[(cj, ci)][:],
                    identity[:],
                    is_transpose=True,
                )

        # ---- softmax: P = exp(m - e) fp8, S = rowsum ----
        pp = [
            pp_pool.tile([128, 2, C], FP8, tag="pp", name=f"pp{s}_{t}")
            for t in range(CB // 2)
        ]
        rs = []
        for ci in range(CB):
            m = small.tile([128, 1], F32, tag="m")
            nc.vector.tensor_reduce(
                out=m[:], in_=e_ps[ci][:], axis=mybir.AxisListType.X,
                op=mybir.AluOpType.min,
            )
            ssum = small.tile([128, 1], F32, tag="s")
            nc.scalar.activation(
                out=pp[ci // 2][:, ci % 2, :],
                in_=e_ps[ci][:],
                func=mybir.ActivationFunctionType.Exp,
                bias=m[:], scale=-1.0, accum_out=ssum[:],
            )
            r = small.tile([128, 1], F32, tag="r")
            nc.vector.reciprocal(out=r[:], in_=ssum[:])
            rs.append(r)

        # rewrite diagonal quadrants: d = (I * (1/S)) * gamma in fp8
        for ci in range(CB):
            nc.vector.tensor_scalar(
                out=dd[ci // 2][:, ci % 2, 128 * (ci % 2) : 128 * (ci % 2) + 128],
                in0=identity[:],
                scalar1=rs[ci][:],
                scalar2=g_sb[:],
                op0=mybir.AluOpType.mult,
                op1=mybir.AluOpType.mult,
            )

        # ---- PT = P.T @ D via DoubleRow: PT[j, i] = gamma * att[i, j] ----
        ptps = [
            psum_g.tile([128, C], F32, tag="g", name=f"ptp{s}_{bj}")
            for bj in range(CB)
        ]
        for t in range(CB // 2):
            for bj in range(CB):
                nc.tensor.matmul(
                    ptps[bj][:, 256 * t : 256 * (t + 1)],
                    pp[t][:, :, 128 * bj : 128 * (bj + 1)],
                    dd[t][:, :, 0:256],
                    start=True,
                    stop=True,
                    perf_mode=DR,
                )
        ptp = [
            ptp_pool.tile([128, 2, C], FP8, tag="pt", name=f"ptp8{s}_{t}")
            for t in range(CB // 2)
        ]
        for bj in range(CB):
            if bj % 2 == 0:
                nc.vector.tensor_copy(out=ptp[bj // 2][:, bj % 2, :], in_=ptps[bj][:])
            else:
                nc.scalar.activation(
                    out=ptp[bj // 2][:, bj % 2, :], in_=ptps[bj][:],
                    func=Copy, bias=0.0, scale=1.0,
                )

        pend.update({"s": s, "m2m": m2m, "ptp": ptp, "nat": nat})

    # ---- tail: the last sample's mm2 ----
    wps = psum_g.tile([128, 128], F32, tag="g", name="wtail")
    for w in range(6):
        nc.tensor.matmul(wps[:], wcon8[:], wcon8[:], start=(w == 0), stop=(w == 5))
    for g in range(CB * NT):
        emit_mm2_group(g, tail=True)
    pend.clear()


_NC_CACHE = None


def _build():
    global _NC_CACHE
    if _NC_CACHE is not None:
        return _NC_CACHE
    from contextlib import ExitStack

    nc = bacc.Bacc("TRN2", target_bir_lowering=False)
    x = nc.dram_tensor("x", [BPC, C, N], F32, kind="ExternalInput")
    gamma = nc.dram_tensor("gamma", [1, 1], F32, kind="ExternalInput")
    out = nc.dram_tensor("out", [BPC, C, N], F16, kind="ExternalOutput")
    with tile.TileContext(nc) as tc:
        with ExitStack() as ctx:
            _emit(nc, tc, ctx, x[:], gamma[:], out[:])
    nc.compile()
    _NC_CACHE = nc
    return nc


def kernel(x, gamma):
    x = np.ascontiguousarray(np.asarray(x, dtype=np.float32))
    gamma = np.ascontiguousarray(np.asarray(gamma, dtype=np.float32))
    assert x.shape == (B, C, H, W), x.shape
    xf = x.reshape(B, C, N)
    nc = _build()
    in_maps = [
        {
            "x": xf[c * BPC : (c + 1) * BPC],
            "gamma": gamma.reshape(1, 1),
        }
        for c in range(NCORES)
    ]
    res = run_bass_kernel_spmd(nc, in_maps, core_ids=list(range(NCORES)))
    out = np.concatenate(
        [np.asarray(res.results[c]["out"]) for c in range(NCORES)], axis=0
    )
    return out.astype(np.float32).reshape(B, C, H, W)
